# revision 22
# baseline (speedup 1.0000x reference)
"""AugmentedConv3D Trainium2 kernel.

Reference computation (B=2, Cin=32, D=H=W=16, DK=32, DV=16, NH=4, KS=3):
  conv_out = conv3d(x, Wc, bc)            # (B, 48, 16,16,16)
  qkv      = conv3d(x, Wqkv, bqkv)        # (B, 80, 16,16,16)
  per head h: logits = (q_h/sqrt(8))^T k_h over P=4096 positions
              attn   = softmax(logits) @ v_h^T        # (P, 4)
  attn reshaped (faithful reshape, not transpose) to (B, 16, D,H,W),
  1x1x1 conv Wo/bo, concat with conv_out on channel axis.

Sharding: one core per (batch b, head h) pair = 8 cores. Each core:
  - fused conv for its batch: output channels [q(8)@0, conv0(24)@8,
    k(8)@32, conv1(24)@40, v(4)@64, ones(1)@68] as ONE K=97 matmul
    accumulation (27 taps = 9 (kd,kh) offsets x (kw,ci)-stacked input
    copies; bias via an all-ones input row; the "ones" channel has zero
    weights and bias 1 and becomes the softmax-denominator column of the
    transposed v).
  - logits^T tiles [keys=128, queries=1024] on PE (f32r), Exp on ACT with
    fused 1/sqrt(8) scale, [v;1]-weighted accumulation on PE -> [5, 4096]
    (rows 0-3 unnormalized attn^T, row 4 = denominator Z).
  - normalizes, regroups for the "faithful reshape", applies its head's
    columns of the 1x1 Wo conv -> partial output [4, 16, 1024].
Host: picks conv_out from one core per batch, sums the 4 head partials
(+bo) per batch — a plain tensor-parallel unshard — and reassembles the
(2, 64, 16, 16, 16) output.
"""
from contextlib import ExitStack

import numpy as np

import concourse.bacc as bacc
import concourse.tile as tile
from concourse import mybir
from concourse.bass_utils import run_bass_kernel_spmd

F32 = mybir.dt.float32
F32R = mybir.dt.float32r
BF16 = mybir.dt.bfloat16

DK, DV, NH, KS = 32, 16, 4, 3
B, CIN, DIM = 2, 32, 16
P = DIM * DIM * DIM            # 4096
DKH, DVH = DK // NH, DV // NH  # 8, 4
NCO = 72                       # conv out channels: q|conv0|k|conv1|v|ones|pad
PAD = DIM + 2                  # 18
SPAT = PAD * PAD * PAD         # 5832
WCC = 9 * NCO                  # 648
XS_OFF = WCC + 16 + 8          # 672
XW_COLS = XS_OFF + SPAT        # 6504
S8 = float(DKH) ** -0.5

_NC_CACHE = []


def _build_module(repeat=1):
    nc = bacc.Bacc("TRN2", target_bir_lowering=False, debug=False, num_devices=8)
    xw = nc.dram_tensor("xw", (128, XW_COLS), F32, kind="ExternalInput").ap()
    conv_out_d = nc.dram_tensor("conv_out", (48, P), F32, kind="ExternalOutput").ap()
    wo_part_d = nc.dram_tensor("wo_part", (4, 16, 1024), F32, kind="ExternalOutput").ap()

    with tile.TileContext(nc) as tc:
      for _rep in range(repeat):
        ctx = ExitStack()
        sbp = ctx.enter_context(tc.tile_pool(name="sb", bufs=1))

        # ---- input: 4 parallel DMAs, then DVE f32 -> f32r casts ----
        phin = ctx.enter_context(tc.tile_pool(name="phin", bufs=1))
        xwf = phin.tile([128, XW_COLS], F32)
        xwr = sbp.tile([128, XW_COLS], F32R)
        qtp = sbp.tile([128, P], BF16)        # q rows 0:8, zeros below (K=128 pad)
        ktp = sbp.tile([128, P], BF16)        # k rows 0:8, zeros below (K=128 pad)
        CH = XW_COLS // 4
        for i in range(4):
            c0, c1 = i * CH, (i + 1) * CH
            nc.sync.dma_start(xwf[:, c0:c1], xw[:, c0:c1])
        for i in range(4):
            c0, c1 = i * CH, (i + 1) * CH
            nc.vector.tensor_copy(xwr[:, c0:c1], xwf[:, c0:c1])
        # zero-fill padded q/k tiles: finite-src * 0 (DVE writes f32r zeros)
        nc.vector.tensor_scalar_mul(qtp[:], xwr[:, 0:P], 0.0)
        nc.vector.tensor_scalar_mul(ktp[:], xwr[:, 0:P], 0.0)

        wconv9 = xwr[:, 0:WCC].rearrange("p (j co) -> p j co", j=9)
        wot = xwr[0:4, WCC:WCC + 16]
        ident8 = xwr[0:8, WCC + 16:WCC + 24]
        xs4 = xwr[:, XS_OFF:XS_OFF + SPAT].rearrange(
            "p (a b c) -> p a b c", a=PAD, b=PAD, c=PAD)

        cstage = sbp.tile([NCO, P], F32R)     # q|conv0|k|conv1|v|ones
        vts = sbp.tile([8, P], F32R)          # v'+ones+pad at base partition 0
        vpt = sbp.tile([128, 32, 8], BF16)    # [m%128, m//128, (v0..v3, 1, 0,0,0)]
        attn5 = sbp.tile([5, P], F32)         # rows 0..3 unnorm attn^T, row 4 = Z
        wos = sbp.tile([16, P], F32)          # Wo partials, r-major columns

        # ---- fused conv + v' transpose + attention (one overlapped phase) ----
        with ExitStack() as phAB:
            cps = phAB.enter_context(tc.tile_pool(name="cps", bufs=1, space="PSUM"))
            vtp = phAB.enter_context(tc.tile_pool(name="vtp", bufs=1, space="PSUM"))
            lgp = phAB.enter_context(tc.tile_pool(name="lg", bufs=2, space="PSUM"))
            o5p = phAB.enter_context(tc.tile_pool(name="o5", bufs=1, space="PSUM"))
            wpl = phAB.enter_context(tc.tile_pool(name="w", bufs=3))
            pvt = vtp.tile([128, 256], F32R)
            for t in range(8):
                cp = cps.tile([NCO, 512], F32)
                for j in range(9):
                    kd, kh = divmod(j, 3)
                    nc.tensor.matmul(
                        cp[:], wconv9[:, j, :],
                        xs4[:, 2 * t + kd:2 * t + kd + 2, kh:kh + DIM, 0:DIM],
                        start=(j == 0), stop=(j == 8))
                sl = np.s_[t * 512:(t + 1) * 512]
                nc.vector.tensor_copy(cstage[:, sl], cp[:])
                nc.vector.tensor_copy(vts[:, sl], cp[64:72])
                for mm in range(4):
                    m = 4 * t + mm
                    nc.tensor.transpose(
                        pvt[:, 8 * m:8 * m + 8],
                        vts[:, m * 128:(m + 1) * 128], ident8)
                nc.vector.tensor_copy(
                    vpt[:, 4 * t:4 * t + 4, :],
                    pvt[:, 32 * t:32 * t + 32].rearrange("p (m c) -> p m c", c=8))
                # q/k slices to padded base-0 tiles, LAST DVE ops of tile t:
                # any logits wait on these shadows this tile's other writes
                nc.vector.tensor_copy(qtp[0:8, sl], cp[0:8])
                nc.vector.tensor_copy(ktp[0:8, sl], cp[32:40])
            nc.sync.dma_start(conv_out_d[0:24], cstage[8:32, :].bitcast(F32))
            nc.sync.dma_start(conv_out_d[24:48], cstage[40:64, :].bitcast(F32))

            for qq in range(4):
                o5 = o5p.tile([5, 1024], F32)
                for m in range(32):
                    lg = lgp.tile([128, 1024], F32)
                    k_ap = ktp[:, m * 128:(m + 1) * 128]
                    nc.tensor.matmul(lg[:, 0:512], k_ap,
                                     qtp[:, qq * 1024:qq * 1024 + 512],
                                     start=True, stop=True)
                    nc.tensor.matmul(lg[:, 512:1024], k_ap,
                                     qtp[:, qq * 1024 + 512:(qq + 1) * 1024],
                                     start=True, stop=True)
                    w = wpl.tile([128, 1024], BF16)
                    nc.scalar.activation(w[:], lg[:],
                                         mybir.ActivationFunctionType.Exp,
                                         scale=S8)
                    vp = vpt[:, m, 0:5]
                    nc.tensor.matmul(o5[:, 0:512], vp, w[:, 0:512],
                                     start=(m == 0), stop=(m == 31))
                    nc.tensor.matmul(o5[:, 512:1024], vp, w[:, 512:1024],
                                     start=(m == 0), stop=(m == 31))
                nc.scalar.copy(attn5[:, qq * 1024:(qq + 1) * 1024], o5[:])

        # ---- phase C: normalize + faithful-reshape regroup + Wo ----
        with ExitStack() as phC:
            pop = phC.enter_context(tc.tile_pool(name="po", bufs=4, space="PSUM"))
            phcs = phC.enter_context(tc.tile_pool(name="phcs", bufs=1))
            # regroup: X_r[j, t] = attn5[r, j*1024+t]  (4KB runs, SBUF->SBUF DMA)
            zd = phcs.tile([4, 1024], F32)
            nc.sync.dma_start(zd[:], attn5[4:5, :])
            zdr = phcs.tile([4, 1024], F32)
            nc.vector.reciprocal(zdr[:], zd[:])
            for r in range(4):
                dr = phcs.tile([4, 1024], F32, tag=f"d{r}")
                nc.sync.dma_start(dr[:], attn5[r:r + 1, :])
                drn = phcs.tile([4, 1024], F32R, tag=f"dn{r}")
                nc.vector.tensor_mul(drn[:], dr[:], zdr[:])
                po = pop.tile([16, 1024], F32)
                nc.tensor.matmul(po[:, 0:512], wot, drn[:, 0:512],
                                 start=True, stop=True)
                nc.tensor.matmul(po[:, 512:1024], wot, drn[:, 512:1024],
                                 start=True, stop=True)
                nc.scalar.copy(wos[:, r * 1024:(r + 1) * 1024], po[:])
                nc.sync.dma_start(wo_part_d[r],
                                  wos[:, r * 1024:(r + 1) * 1024])
        ctx.close()

    nc.compile()
    return nc


def _build_null_module():
    """Tiny do-nothing module used by test.py to measure dispatch overhead."""
    nc = bacc.Bacc("TRN2", target_bir_lowering=False, debug=False, num_devices=8)
    nin = nc.dram_tensor("nin", (1, 16), F32, kind="ExternalInput").ap()
    nout = nc.dram_tensor("nout", (1, 16), F32, kind="ExternalOutput").ap()
    with tile.TileContext(nc) as tc, ExitStack() as ctx:
        p = ctx.enter_context(tc.tile_pool(name="p", bufs=1))
        t = p.tile([1, 16], F32)
        nc.sync.dma_start(t[:], nin[:])
        t2 = p.tile([1, 16], F32)
        nc.vector.tensor_copy(t2[:], t[:])
        nc.sync.dma_start(nout[:], t2[:])
    nc.compile()
    return nc


def _prep_core_input(x, Wc, bc, Wqkv, bqkv, Wo, b, h):
    """Build the [97, XW_COLS] f32 input blob for core (b, h)."""
    xpad = np.zeros((CIN, PAD, PAD, PAD), np.float32)
    xpad[:, 1:17, 1:17, 1:17] = x[b]
    flat = xpad.reshape(CIN, SPAT)
    xs = np.zeros((128, SPAT), np.float32)
    for kw in range(3):
        xs[kw * 32:(kw + 1) * 32, 0:SPAT - kw] = flat[:, kw:]
    xs[96] = 1.0

    Wsel = np.zeros((NCO, CIN, 3, 3, 3), np.float32)
    bsel = np.zeros((NCO,), np.float32)
    Wsel[0:8] = Wqkv[h * 8:(h + 1) * 8]
    bsel[0:8] = bqkv[h * 8:(h + 1) * 8]
    Wsel[8:32] = Wc[0:24]
    bsel[8:32] = bc[0:24]
    Wsel[32:40] = Wqkv[DK + h * 8:DK + (h + 1) * 8]
    bsel[32:40] = bqkv[DK + h * 8:DK + (h + 1) * 8]
    Wsel[40:64] = Wc[24:48]
    bsel[40:64] = bc[24:48]
    Wsel[64:68] = Wqkv[2 * DK + h * 4:2 * DK + (h + 1) * 4]
    bsel[64:68] = bqkv[2 * DK + h * 4:2 * DK + (h + 1) * 4]
    bsel[68] = 1.0                               # the ones channel
    # [kw*32+ci, kd*3+kh, co]
    w9 = Wsel.transpose(4, 1, 2, 3, 0).reshape(96, 9, NCO)
    wconv = np.zeros((128, 9, NCO), np.float32)
    wconv[0:96] = w9
    wconv[96, 0, :] = bsel

    Wo16 = Wo[:, :, 0, 0, 0]                      # [16, 16]
    wot = np.ascontiguousarray(Wo16[:, 4 * h:4 * h + 4].T)  # [4, 16]

    xw = np.zeros((128, XW_COLS), np.float32)
    xw[:, 0:WCC] = wconv.reshape(128, WCC)
    xw[0:4, WCC:WCC + 16] = wot
    xw[0:8, WCC + 16:WCC + 24] = np.eye(8, dtype=np.float32)
    xw[:, XS_OFF:XS_OFF + SPAT] = xs
    return xw


def kernel(x, Wc, bc, Wqkv, bqkv, Wo, bo):
    x = np.asarray(x, np.float32)
    Wc = np.asarray(Wc, np.float32)
    bc = np.asarray(bc, np.float32)
    Wqkv = np.asarray(Wqkv, np.float32)
    bqkv = np.asarray(bqkv, np.float32)
    Wo = np.asarray(Wo, np.float32)
    bo = np.asarray(bo, np.float32)

    if not _NC_CACHE:
        _NC_CACHE.append(_build_module())
    nc = _NC_CACHE[0]

    in_maps = [
        {"xw": _prep_core_input(x, Wc, bc, Wqkv, bqkv, Wo, c // 4, c % 4)}
        for c in range(8)
    ]
    res = run_bass_kernel_spmd(nc, in_maps, core_ids=list(range(8)))

    out = np.empty((B, 64, DIM, DIM, DIM), np.float32)
    for b in range(B):
        out[b, 0:48] = res.results[4 * b]["conv_out"].reshape(48, DIM, DIM, DIM)
        acc = np.zeros((16, P), np.float32)
        for h in range(NH):
            wp = res.results[4 * b + h]["wo_part"]      # [4, 16, 1024]
            acc += wp.transpose(1, 2, 0).reshape(16, P)
        acc += bo[:, None]
        out[b, 48:64] = acc.reshape(16, DIM, DIM, DIM)
    return out


# revision 23
# speedup vs baseline: 1.0077x; 1.0077x over previous
"""AugmentedConv3D Trainium2 kernel.

Reference computation (B=2, Cin=32, D=H=W=16, DK=32, DV=16, NH=4, KS=3):
  conv_out = conv3d(x, Wc, bc)            # (B, 48, 16,16,16)
  qkv      = conv3d(x, Wqkv, bqkv)        # (B, 80, 16,16,16)
  per head h: logits = (q_h/sqrt(8))^T k_h over P=4096 positions
              attn   = softmax(logits) @ v_h^T        # (P, 4)
  attn reshaped (faithful reshape, not transpose) to (B, 16, D,H,W),
  1x1x1 conv Wo/bo, concat with conv_out on channel axis.

Sharding: one core per (batch b, head h) pair = 8 cores. Each core:
  - fused conv for its batch: output channels [q(8)@0, conv0(24)@8,
    k(8)@32, conv1(24)@40, v(4)@64, ones(1)@68] as ONE K=97 matmul
    accumulation (27 taps = 9 (kd,kh) offsets x (kw,ci)-stacked input
    copies; bias via an all-ones input row; the "ones" channel has zero
    weights and bias 1 and becomes the softmax-denominator column of the
    transposed v).
  - logits^T tiles [keys=128, queries=1024] on PE (f32r), Exp on ACT with
    fused 1/sqrt(8) scale, [v;1]-weighted accumulation on PE -> [5, 4096]
    (rows 0-3 unnormalized attn^T, row 4 = denominator Z).
  - normalizes, regroups for the "faithful reshape", applies its head's
    columns of the 1x1 Wo conv -> partial output [4, 16, 1024].
Host: picks conv_out from one core per batch, sums the 4 head partials
(+bo) per batch — a plain tensor-parallel unshard — and reassembles the
(2, 64, 16, 16, 16) output.
"""
from contextlib import ExitStack

import numpy as np

import concourse.bacc as bacc
import concourse.tile as tile
from concourse import mybir
from concourse.bass_utils import run_bass_kernel_spmd

F32 = mybir.dt.float32
F32R = mybir.dt.float32r

DK, DV, NH, KS = 32, 16, 4, 3
B, CIN, DIM = 2, 32, 16
P = DIM * DIM * DIM            # 4096
DKH, DVH = DK // NH, DV // NH  # 8, 4
NCO = 72                       # conv out channels: q|conv0|k|conv1|v|ones|pad
PAD = DIM + 2                  # 18
SPAT = PAD * PAD * PAD         # 5832
WCC = 9 * NCO                  # 648
XS_OFF = WCC + 16 + 8          # 672
XW_COLS = XS_OFF + SPAT        # 6504
S8 = float(DKH) ** -0.5

_NC_CACHE = []


def _build_module(repeat=1):
    nc = bacc.Bacc("TRN2", target_bir_lowering=False, debug=False, num_devices=8)
    xw = nc.dram_tensor("xw", (128, XW_COLS), F32, kind="ExternalInput").ap()
    conv_out_d = nc.dram_tensor("conv_out", (48, P), F32, kind="ExternalOutput").ap()
    wo_part_d = nc.dram_tensor("wo_part", (4, 16, 1024), F32, kind="ExternalOutput").ap()

    with tile.TileContext(nc) as tc:
      for _rep in range(repeat):
        ctx = ExitStack()
        sbp = ctx.enter_context(tc.tile_pool(name="sb", bufs=1))

        # ---- input: 4 parallel DMAs, then DVE f32 -> f32r casts ----
        phin = ctx.enter_context(tc.tile_pool(name="phin", bufs=1))
        xwf = phin.tile([128, XW_COLS], F32)
        xwr = sbp.tile([128, XW_COLS], F32R)
        qtp = sbp.tile([128, P], F32R)        # q rows 0:8, zeros below (K=128 pad)
        ktp = sbp.tile([128, P], F32R)        # k rows 0:8, zeros below (K=128 pad)
        CH = XW_COLS // 4
        for i in range(4):
            c0, c1 = i * CH, (i + 1) * CH
            nc.sync.dma_start(xwf[:, c0:c1], xw[:, c0:c1])
        for i in range(4):
            c0, c1 = i * CH, (i + 1) * CH
            nc.vector.tensor_copy(xwr[:, c0:c1], xwf[:, c0:c1])
        # zero-fill padded q/k tiles: finite-src * 0 (DVE writes f32r zeros)
        nc.vector.tensor_scalar_mul(qtp[:], xwr[:, 0:P], 0.0)
        nc.vector.tensor_scalar_mul(ktp[:], xwr[:, 0:P], 0.0)

        wconv9 = xwr[:, 0:WCC].rearrange("p (j co) -> p j co", j=9)
        wot = xwr[0:4, WCC:WCC + 16]
        ident8 = xwr[0:8, WCC + 16:WCC + 24]
        xs4 = xwr[:, XS_OFF:XS_OFF + SPAT].rearrange(
            "p (a b c) -> p a b c", a=PAD, b=PAD, c=PAD)

        cstage = sbp.tile([NCO, P], F32R)     # q|conv0|k|conv1|v|ones
        vts = sbp.tile([8, P], F32R)          # v'+ones+pad at base partition 0
        vpt = sbp.tile([128, 32, 8], F32R)    # [m%128, m//128, (v0..v3, 1, 0,0,0)]
        attn5 = sbp.tile([5, P], F32)         # rows 0..3 unnorm attn^T, row 4 = Z
        wos = sbp.tile([16, P], F32)          # Wo partials, r-major columns

        # ---- fused conv + v' transpose + attention (one overlapped phase) ----
        with ExitStack() as phAB:
            cps = phAB.enter_context(tc.tile_pool(name="cps", bufs=1, space="PSUM"))
            vtp = phAB.enter_context(tc.tile_pool(name="vtp", bufs=1, space="PSUM"))
            lgp = phAB.enter_context(tc.tile_pool(name="lg", bufs=2, space="PSUM"))
            o5p = phAB.enter_context(tc.tile_pool(name="o5", bufs=1, space="PSUM"))
            wpl = phAB.enter_context(tc.tile_pool(name="w", bufs=3))
            pvt = vtp.tile([128, 256], F32R)
            for t in range(8):
                cp = cps.tile([NCO, 512], F32)
                for j in range(9):
                    kd, kh = divmod(j, 3)
                    nc.tensor.matmul(
                        cp[:], wconv9[:, j, :],
                        xs4[:, 2 * t + kd:2 * t + kd + 2, kh:kh + DIM, 0:DIM],
                        start=(j == 0), stop=(j == 8))
                sl = np.s_[t * 512:(t + 1) * 512]
                nc.vector.tensor_copy(cstage[:, sl], cp[:])
                nc.vector.tensor_copy(vts[:, sl], cp[64:72])
                for mm in range(4):
                    m = 4 * t + mm
                    nc.tensor.transpose(
                        pvt[:, 8 * m:8 * m + 8],
                        vts[:, m * 128:(m + 1) * 128], ident8)
                nc.vector.tensor_copy(
                    vpt[:, 4 * t:4 * t + 4, :],
                    pvt[:, 32 * t:32 * t + 32].rearrange("p (m c) -> p m c", c=8))
                # q/k slices to padded base-0 tiles, LAST DVE ops of tile t:
                # any logits wait on these shadows this tile's other writes
                nc.vector.tensor_copy(qtp[0:8, sl], cp[0:8])
                nc.vector.tensor_copy(ktp[0:8, sl], cp[32:40])
            nc.sync.dma_start(conv_out_d[0:24], cstage[8:32, :].bitcast(F32))
            nc.sync.dma_start(conv_out_d[24:48], cstage[40:64, :].bitcast(F32))

            for qq in range(4):
                o5 = o5p.tile([5, 1024], F32)
                for m in range(32):
                    lg = lgp.tile([128, 1024], F32)
                    k_ap = ktp[:, m * 128:(m + 1) * 128]
                    nc.tensor.matmul(lg[:, 0:512], k_ap,
                                     qtp[:, qq * 1024:qq * 1024 + 512],
                                     start=True, stop=True)
                    nc.tensor.matmul(lg[:, 512:1024], k_ap,
                                     qtp[:, qq * 1024 + 512:(qq + 1) * 1024],
                                     start=True, stop=True)
                    w = wpl.tile([128, 1024], F32R)
                    nc.scalar.activation(w[:], lg[:],
                                         mybir.ActivationFunctionType.Exp,
                                         scale=S8)
                    vp = vpt[:, m, 0:5]
                    nc.tensor.matmul(o5[:, 0:512], vp, w[:, 0:512],
                                     start=(m == 0), stop=(m == 31))
                    nc.tensor.matmul(o5[:, 512:1024], vp, w[:, 512:1024],
                                     start=(m == 0), stop=(m == 31))
                nc.scalar.copy(attn5[:, qq * 1024:(qq + 1) * 1024], o5[:])

        # ---- phase C: normalize + faithful-reshape regroup + Wo ----
        with ExitStack() as phC:
            pop = phC.enter_context(tc.tile_pool(name="po", bufs=4, space="PSUM"))
            phcs = phC.enter_context(tc.tile_pool(name="phcs", bufs=1))
            # regroup: X_r[j, t] = attn5[r, j*1024+t]  (4KB runs, SBUF->SBUF DMA)
            zd = phcs.tile([4, 1024], F32)
            nc.sync.dma_start(zd[:], attn5[4:5, :])
            zdr = phcs.tile([4, 1024], F32)
            nc.vector.reciprocal(zdr[:], zd[:])
            for r in range(4):
                dr = phcs.tile([4, 1024], F32, tag=f"d{r}")
                nc.sync.dma_start(dr[:], attn5[r:r + 1, :])
                drn = phcs.tile([4, 1024], F32R, tag=f"dn{r}")
                nc.vector.tensor_mul(drn[:], dr[:], zdr[:])
                po = pop.tile([16, 1024], F32)
                nc.tensor.matmul(po[:, 0:512], wot, drn[:, 0:512],
                                 start=True, stop=True)
                nc.tensor.matmul(po[:, 512:1024], wot, drn[:, 512:1024],
                                 start=True, stop=True)
                nc.scalar.copy(wos[:, r * 1024:(r + 1) * 1024], po[:])
                nc.sync.dma_start(wo_part_d[r],
                                  wos[:, r * 1024:(r + 1) * 1024])
        ctx.close()

    nc.compile()
    return nc


def _build_null_module():
    """Tiny do-nothing module used by test.py to measure dispatch overhead."""
    nc = bacc.Bacc("TRN2", target_bir_lowering=False, debug=False, num_devices=8)
    nin = nc.dram_tensor("nin", (1, 16), F32, kind="ExternalInput").ap()
    nout = nc.dram_tensor("nout", (1, 16), F32, kind="ExternalOutput").ap()
    with tile.TileContext(nc) as tc, ExitStack() as ctx:
        p = ctx.enter_context(tc.tile_pool(name="p", bufs=1))
        t = p.tile([1, 16], F32)
        nc.sync.dma_start(t[:], nin[:])
        t2 = p.tile([1, 16], F32)
        nc.vector.tensor_copy(t2[:], t[:])
        nc.sync.dma_start(nout[:], t2[:])
    nc.compile()
    return nc


def _prep_core_input(x, Wc, bc, Wqkv, bqkv, Wo, b, h):
    """Build the [97, XW_COLS] f32 input blob for core (b, h)."""
    xpad = np.zeros((CIN, PAD, PAD, PAD), np.float32)
    xpad[:, 1:17, 1:17, 1:17] = x[b]
    flat = xpad.reshape(CIN, SPAT)
    xs = np.zeros((128, SPAT), np.float32)
    for kw in range(3):
        xs[kw * 32:(kw + 1) * 32, 0:SPAT - kw] = flat[:, kw:]
    xs[96] = 1.0

    Wsel = np.zeros((NCO, CIN, 3, 3, 3), np.float32)
    bsel = np.zeros((NCO,), np.float32)
    Wsel[0:8] = Wqkv[h * 8:(h + 1) * 8]
    bsel[0:8] = bqkv[h * 8:(h + 1) * 8]
    Wsel[8:32] = Wc[0:24]
    bsel[8:32] = bc[0:24]
    Wsel[32:40] = Wqkv[DK + h * 8:DK + (h + 1) * 8]
    bsel[32:40] = bqkv[DK + h * 8:DK + (h + 1) * 8]
    Wsel[40:64] = Wc[24:48]
    bsel[40:64] = bc[24:48]
    Wsel[64:68] = Wqkv[2 * DK + h * 4:2 * DK + (h + 1) * 4]
    bsel[64:68] = bqkv[2 * DK + h * 4:2 * DK + (h + 1) * 4]
    bsel[68] = 1.0                               # the ones channel
    # [kw*32+ci, kd*3+kh, co]
    w9 = Wsel.transpose(4, 1, 2, 3, 0).reshape(96, 9, NCO)
    wconv = np.zeros((128, 9, NCO), np.float32)
    wconv[0:96] = w9
    wconv[96, 0, :] = bsel

    Wo16 = Wo[:, :, 0, 0, 0]                      # [16, 16]
    wot = np.ascontiguousarray(Wo16[:, 4 * h:4 * h + 4].T)  # [4, 16]

    xw = np.zeros((128, XW_COLS), np.float32)
    xw[:, 0:WCC] = wconv.reshape(128, WCC)
    xw[0:4, WCC:WCC + 16] = wot
    xw[0:8, WCC + 16:WCC + 24] = np.eye(8, dtype=np.float32)
    xw[:, XS_OFF:XS_OFF + SPAT] = xs
    return xw


def kernel(x, Wc, bc, Wqkv, bqkv, Wo, bo):
    x = np.asarray(x, np.float32)
    Wc = np.asarray(Wc, np.float32)
    bc = np.asarray(bc, np.float32)
    Wqkv = np.asarray(Wqkv, np.float32)
    bqkv = np.asarray(bqkv, np.float32)
    Wo = np.asarray(Wo, np.float32)
    bo = np.asarray(bo, np.float32)

    if not _NC_CACHE:
        _NC_CACHE.append(_build_module())
    nc = _NC_CACHE[0]

    in_maps = [
        {"xw": _prep_core_input(x, Wc, bc, Wqkv, bqkv, Wo, c // 4, c % 4)}
        for c in range(8)
    ]
    res = run_bass_kernel_spmd(nc, in_maps, core_ids=list(range(8)))

    out = np.empty((B, 64, DIM, DIM, DIM), np.float32)
    for b in range(B):
        out[b, 0:48] = res.results[4 * b]["conv_out"].reshape(48, DIM, DIM, DIM)
        acc = np.zeros((16, P), np.float32)
        for h in range(NH):
            wp = res.results[4 * b + h]["wo_part"]      # [4, 16, 1024]
            acc += wp.transpose(1, 2, 0).reshape(16, P)
        acc += bo[:, None]
        out[b, 48:64] = acc.reshape(16, DIM, DIM, DIM)
    return out


# revision 24
# speedup vs baseline: 1.4577x; 1.4465x over previous
"""AugmentedConv3D Trainium2 kernel.

Reference computation (B=2, Cin=32, D=H=W=16, DK=32, DV=16, NH=4, KS=3):
  conv_out = conv3d(x, Wc, bc)            # (B, 48, 16,16,16)
  qkv      = conv3d(x, Wqkv, bqkv)        # (B, 80, 16,16,16)
  per head h: logits = (q_h/sqrt(8))^T k_h over P=4096 positions
              attn   = softmax(logits) @ v_h^T        # (P, 4)
  attn reshaped (faithful reshape, not transpose) to (B, 16, D,H,W),
  1x1x1 conv Wo/bo, concat with conv_out on channel axis.

Sharding: one core per (batch b, head h) pair = 8 cores. Each core:
  - fused conv for its batch: output channels [q(8)@0, conv0(24)@8,
    k(8)@32, conv1(24)@40, v(4)@64, ones(1)@68] as ONE K=97 matmul
    accumulation (27 taps = 9 (kd,kh) offsets x (kw,ci)-stacked input
    copies; bias via an all-ones input row; the "ones" channel has zero
    weights and bias 1 and becomes the softmax-denominator column of the
    transposed v).
  - logits^T tiles [keys=128, queries=1024] on PE (f32r), Exp on ACT with
    fused 1/sqrt(8) scale, [v;1]-weighted accumulation on PE -> [5, 4096]
    (rows 0-3 unnormalized attn^T, row 4 = denominator Z).
  - normalizes, regroups for the "faithful reshape", applies its head's
    columns of the 1x1 Wo conv -> partial output [4, 16, 1024].
Host: picks conv_out from one core per batch, sums the 4 head partials
(+bo) per batch — a plain tensor-parallel unshard — and reassembles the
(2, 64, 16, 16, 16) output.
"""
from contextlib import ExitStack

import numpy as np

import concourse.bacc as bacc
import concourse.tile as tile
from concourse import mybir
from concourse.bass_utils import run_bass_kernel_spmd

F32 = mybir.dt.float32
F32R = mybir.dt.float32r

DK, DV, NH, KS = 32, 16, 4, 3
B, CIN, DIM = 2, 32, 16
P = DIM * DIM * DIM            # 4096
DKH, DVH = DK // NH, DV // NH  # 8, 4
NCO = 72                       # conv out channels: q|conv0|k|conv1|v|ones|pad
PAD = DIM + 2                  # 18
SPAT = PAD * PAD * PAD         # 5832
WCC = 9 * NCO                  # 648
XS_OFF = WCC + 16 + 8          # 672
XW_COLS = XS_OFF + SPAT        # 6504
S8 = float(DKH) ** -0.5

_NC_CACHE = []


def _build_module(repeat=1):
    nc = bacc.Bacc("TRN2", target_bir_lowering=False, debug=False, num_devices=8)
    xw = nc.dram_tensor("xw", (128, XW_COLS), F32, kind="ExternalInput").ap()
    conv_out_d = nc.dram_tensor("conv_out", (48, P), F32, kind="ExternalOutput").ap()
    wo_part_d = nc.dram_tensor("wo_part", (4, 16, 1024), F32, kind="ExternalOutput").ap()

    with tile.TileContext(nc) as tc:
      for _rep in range(repeat):
        ctx = ExitStack()
        sbp = ctx.enter_context(tc.tile_pool(name="sb", bufs=1))

        # ---- input: 4 parallel DMAs, then DVE f32 -> f32r casts ----
        phin = ctx.enter_context(tc.tile_pool(name="phin", bufs=1))
        xwf = phin.tile([128, XW_COLS], F32)
        xwr = sbp.tile([128, XW_COLS], F32R)
        qtp = sbp.tile([128, P], F32R)        # q rows 0:8, zeros below (K=128 pad)
        ktp = sbp.tile([128, P], F32R)        # k rows 0:8, zeros below (K=128 pad)
        CH = XW_COLS // 4
        for i in range(4):
            c0, c1 = i * CH, (i + 1) * CH
            nc.sync.dma_start(xwf[:, c0:c1], xw[:, c0:c1])
        for i in range(4):
            c0, c1 = i * CH, (i + 1) * CH
            nc.vector.tensor_copy(xwr[:, c0:c1], xwf[:, c0:c1])
        # zero-fill padded q/k tiles: finite-src * 0 (DVE writes f32r zeros)
        nc.vector.tensor_scalar_mul(qtp[:], xwr[:, 0:P], 0.0)
        nc.vector.tensor_scalar_mul(ktp[:], xwr[:, 0:P], 0.0)

        wconv9 = xwr[:, 0:WCC].rearrange("p (j co) -> p j co", j=9)
        wot = xwr[0:4, WCC:WCC + 16]
        ident8 = xwr[0:8, WCC + 16:WCC + 24]
        xs4 = xwr[:, XS_OFF:XS_OFF + SPAT].rearrange(
            "p (a b c) -> p a b c", a=PAD, b=PAD, c=PAD)

        cstage = sbp.tile([NCO, P], F32R)     # q|conv0|k|conv1|v|ones
        vts = sbp.tile([8, P], F32R)          # v'+ones+pad at base partition 0
        vpt = sbp.tile([128, 32, 8], F32R)    # [m%128, m//128, (v0..v3, 1, 0,0,0)]
        attn5 = sbp.tile([5, P], F32)         # rows 0..3 unnorm attn^T, row 4 = Z
        wos = sbp.tile([16, P], F32)          # Wo partials, r-major columns

        # ---- fused conv + v' transpose + attention (one overlapped phase) ----
        with ExitStack() as phAB:
            cps = phAB.enter_context(tc.tile_pool(name="cps", bufs=1, space="PSUM"))
            vtp = phAB.enter_context(tc.tile_pool(name="vtp", bufs=1, space="PSUM"))
            lgp = phAB.enter_context(tc.tile_pool(name="lg", bufs=4, space="PSUM"))
            o5p = phAB.enter_context(tc.tile_pool(name="o5", bufs=1, space="PSUM"))
            wpl = phAB.enter_context(tc.tile_pool(name="w", bufs=4))
            pvt = vtp.tile([128, 256], F32R)
            for t in range(8):
                cp = cps.tile([NCO, 512], F32)
                for j in range(9):
                    kd, kh = divmod(j, 3)
                    nc.tensor.matmul(
                        cp[:], wconv9[:, j, :],
                        xs4[:, 2 * t + kd:2 * t + kd + 2, kh:kh + DIM, 0:DIM],
                        start=(j == 0), stop=(j == 8))
                sl = np.s_[t * 512:(t + 1) * 512]
                nc.vector.tensor_copy(cstage[:, sl], cp[:])
                nc.vector.tensor_copy(vts[:, sl], cp[64:72])
                for mm in range(4):
                    m = 4 * t + mm
                    nc.tensor.transpose(
                        pvt[:, 8 * m:8 * m + 8],
                        vts[:, m * 128:(m + 1) * 128], ident8)
                nc.vector.tensor_copy(
                    vpt[:, 4 * t:4 * t + 4, :],
                    pvt[:, 32 * t:32 * t + 32].rearrange("p (m c) -> p m c", c=8))
                # q/k slices to padded base-0 tiles, LAST DVE ops of tile t:
                # any logits wait on these shadows this tile's other writes
                nc.vector.tensor_copy(qtp[0:8, sl], cp[0:8])
                nc.vector.tensor_copy(ktp[0:8, sl], cp[32:40])
            nc.sync.dma_start(conv_out_d[0:24], cstage[8:32, :].bitcast(F32))
            nc.sync.dma_start(conv_out_d[24:48], cstage[40:64, :].bitcast(F32))

            for qq in range(4):
                o5 = o5p.tile([5, 1024], F32)
                for m in range(32):
                    k_ap = ktp[:, m * 128:(m + 1) * 128]
                    vp = vpt[:, m, 0:5]
                    for hh in range(2):
                        c0 = qq * 1024 + hh * 512
                        lg = lgp.tile([128, 512], F32)
                        nc.tensor.matmul(lg[:], k_ap, qtp[:, c0:c0 + 512],
                                         start=True, stop=True)
                        w = wpl.tile([128, 512], F32R)
                        nc.scalar.activation(w[:], lg[:],
                                             mybir.ActivationFunctionType.Exp,
                                             scale=S8)
                        nc.tensor.matmul(o5[:, hh * 512:hh * 512 + 512], vp, w[:],
                                         start=(m == 0), stop=(m == 31))
                nc.scalar.copy(attn5[:, qq * 1024:(qq + 1) * 1024], o5[:])

        # ---- phase C: normalize + faithful-reshape regroup + Wo ----
        with ExitStack() as phC:
            pop = phC.enter_context(tc.tile_pool(name="po", bufs=4, space="PSUM"))
            phcs = phC.enter_context(tc.tile_pool(name="phcs", bufs=1))
            # regroup: X_r[j, t] = attn5[r, j*1024+t]  (4KB runs, SBUF->SBUF DMA)
            zd = phcs.tile([4, 1024], F32)
            nc.sync.dma_start(zd[:], attn5[4:5, :])
            zdr = phcs.tile([4, 1024], F32)
            nc.vector.reciprocal(zdr[:], zd[:])
            for r in range(4):
                dr = phcs.tile([4, 1024], F32, tag=f"d{r}")
                nc.sync.dma_start(dr[:], attn5[r:r + 1, :])
                drn = phcs.tile([4, 1024], F32R, tag=f"dn{r}")
                nc.vector.tensor_mul(drn[:], dr[:], zdr[:])
                po = pop.tile([16, 1024], F32)
                nc.tensor.matmul(po[:, 0:512], wot, drn[:, 0:512],
                                 start=True, stop=True)
                nc.tensor.matmul(po[:, 512:1024], wot, drn[:, 512:1024],
                                 start=True, stop=True)
                nc.scalar.copy(wos[:, r * 1024:(r + 1) * 1024], po[:])
                nc.sync.dma_start(wo_part_d[r],
                                  wos[:, r * 1024:(r + 1) * 1024])
        ctx.close()

    nc.compile()
    return nc


def _build_null_module():
    """Tiny do-nothing module used by test.py to measure dispatch overhead."""
    nc = bacc.Bacc("TRN2", target_bir_lowering=False, debug=False, num_devices=8)
    nin = nc.dram_tensor("nin", (1, 16), F32, kind="ExternalInput").ap()
    nout = nc.dram_tensor("nout", (1, 16), F32, kind="ExternalOutput").ap()
    with tile.TileContext(nc) as tc, ExitStack() as ctx:
        p = ctx.enter_context(tc.tile_pool(name="p", bufs=1))
        t = p.tile([1, 16], F32)
        nc.sync.dma_start(t[:], nin[:])
        t2 = p.tile([1, 16], F32)
        nc.vector.tensor_copy(t2[:], t[:])
        nc.sync.dma_start(nout[:], t2[:])
    nc.compile()
    return nc


def _prep_core_input(x, Wc, bc, Wqkv, bqkv, Wo, b, h):
    """Build the [97, XW_COLS] f32 input blob for core (b, h)."""
    xpad = np.zeros((CIN, PAD, PAD, PAD), np.float32)
    xpad[:, 1:17, 1:17, 1:17] = x[b]
    flat = xpad.reshape(CIN, SPAT)
    xs = np.zeros((128, SPAT), np.float32)
    for kw in range(3):
        xs[kw * 32:(kw + 1) * 32, 0:SPAT - kw] = flat[:, kw:]
    xs[96] = 1.0

    Wsel = np.zeros((NCO, CIN, 3, 3, 3), np.float32)
    bsel = np.zeros((NCO,), np.float32)
    Wsel[0:8] = Wqkv[h * 8:(h + 1) * 8]
    bsel[0:8] = bqkv[h * 8:(h + 1) * 8]
    Wsel[8:32] = Wc[0:24]
    bsel[8:32] = bc[0:24]
    Wsel[32:40] = Wqkv[DK + h * 8:DK + (h + 1) * 8]
    bsel[32:40] = bqkv[DK + h * 8:DK + (h + 1) * 8]
    Wsel[40:64] = Wc[24:48]
    bsel[40:64] = bc[24:48]
    Wsel[64:68] = Wqkv[2 * DK + h * 4:2 * DK + (h + 1) * 4]
    bsel[64:68] = bqkv[2 * DK + h * 4:2 * DK + (h + 1) * 4]
    bsel[68] = 1.0                               # the ones channel
    # [kw*32+ci, kd*3+kh, co]
    w9 = Wsel.transpose(4, 1, 2, 3, 0).reshape(96, 9, NCO)
    wconv = np.zeros((128, 9, NCO), np.float32)
    wconv[0:96] = w9
    wconv[96, 0, :] = bsel

    Wo16 = Wo[:, :, 0, 0, 0]                      # [16, 16]
    wot = np.ascontiguousarray(Wo16[:, 4 * h:4 * h + 4].T)  # [4, 16]

    xw = np.zeros((128, XW_COLS), np.float32)
    xw[:, 0:WCC] = wconv.reshape(128, WCC)
    xw[0:4, WCC:WCC + 16] = wot
    xw[0:8, WCC + 16:WCC + 24] = np.eye(8, dtype=np.float32)
    xw[:, XS_OFF:XS_OFF + SPAT] = xs
    return xw


def kernel(x, Wc, bc, Wqkv, bqkv, Wo, bo):
    x = np.asarray(x, np.float32)
    Wc = np.asarray(Wc, np.float32)
    bc = np.asarray(bc, np.float32)
    Wqkv = np.asarray(Wqkv, np.float32)
    bqkv = np.asarray(bqkv, np.float32)
    Wo = np.asarray(Wo, np.float32)
    bo = np.asarray(bo, np.float32)

    if not _NC_CACHE:
        _NC_CACHE.append(_build_module())
    nc = _NC_CACHE[0]

    in_maps = [
        {"xw": _prep_core_input(x, Wc, bc, Wqkv, bqkv, Wo, c // 4, c % 4)}
        for c in range(8)
    ]
    res = run_bass_kernel_spmd(nc, in_maps, core_ids=list(range(8)))

    out = np.empty((B, 64, DIM, DIM, DIM), np.float32)
    for b in range(B):
        out[b, 0:48] = res.results[4 * b]["conv_out"].reshape(48, DIM, DIM, DIM)
        acc = np.zeros((16, P), np.float32)
        for h in range(NH):
            wp = res.results[4 * b + h]["wo_part"]      # [4, 16, 1024]
            acc += wp.transpose(1, 2, 0).reshape(16, P)
        acc += bo[:, None]
        out[b, 48:64] = acc.reshape(16, DIM, DIM, DIM)
    return out
